# revision 18
# baseline (speedup 1.0000x reference)
"""Bass/Trainium2 kernel for batched dot-product attention.

Problem: q,k,v [B=4, S=4096, D=1024]; projections to dk=dv=128; softmax
attention per batch element.  Sharded over 8 NeuronCores as (batch,
key-half): core c handles batch c//2, keys (c%2)*2048 ... +2048, ALL
4096 queries.  Each core emits a partial numerator (out^T [dv, q]) and
partial denominator (sums [1, q]); the host combines the two halves and
normalizes: out = (oA+oB)/(sA+sB).

On-chip layouts keep the contraction dim on SBUF partitions:
  qT/kT/vT  [d_model, seq]    (host pre-transposed; q/k fp8, v bf16)
  qpT/kpT   [dk, seq]  bf16   (projection outputs)
  vp        [keys, dv] bf16   (PE transpose)
  S^T       [keys(128), q]    PSUM f32, 2 key-chunks per 2048-wide tile
  exp       [keys, 2*1024q]   SBUF bf16, one ScalarE op per 2 chunks
  out^T     [dv, q]           PSUM f32 accumulated over key chunks

Q/K projections run fp8 e4m3 with DoubleRow (2 d_model-chunks per
matmul); weights are host-scaled by 64 into fp8's normal range and the
bias-add stage rescales.  The softmax denominator: VectorE adds the two
chunk-halves of each exp tile (bf16, 2x mode), GpSimd accumulates f32,
and a ones-vector matmul reduces over the key partition axis at the end
of each pair.  exp() uses no max-subtraction (scores are O(1)).  AV
matmuls drain from a deep exp-tile buffer whenever the PE has slack, so
the PE never waits on ScalarE's exp stream.
"""

import math

import numpy as np
import ml_dtypes

import concourse.bass as bass
import concourse.tile as tile
from concourse import bacc, mybir
from concourse.bass_utils import run_bass_kernel_spmd

B, S, DM, DK, DV = 4, 4096, 1024, 128, 128
N_CORES = 8
SK = S // 2            # keys per core
NKB = SK // 512        # key blocks per core (4)
NKC = SK // 128        # key chunks of 128 (16)
NQB = S // 512         # query blocks (8)
NPAIR = NQB // 2       # query pairs of 1024 (4)
NMC = DM // 128        # d_model chunks (8)
NSG = NKC // 2         # score groups (2 key chunks) per pair (8)

USE_FP8 = True

BF16 = mybir.dt.bfloat16
F32 = mybir.dt.float32
FP8 = mybir.dt.float8e4
NP_BF16 = ml_dtypes.bfloat16
NP_FP8 = ml_dtypes.float8_e4m3

W_SCALE = 64.0                       # fp8 weight pre-scale (host)
SC = 1.0 / math.sqrt(DK)             # attention score scale

X_DT = FP8 if USE_FP8 else BF16      # q/k input + weight dtype
NP_X = NP_FP8 if USE_FP8 else NP_BF16

AV_LAG = 5                           # chunks the AV drain trails by

Exp = mybir.ActivationFunctionType.Exp
DoubleRow = mybir.MatmulPerfMode.DoubleRow


def _emit(tc: tile.TileContext, aps: dict):
    nc = tc.nc

    with tc.tile_pool(name="persist", bufs=1) as persist:
        # --- constants ---
        # constants packed into wide-row tensors: tiny per-partition rows
        # (16-256B) pay full HBM transaction latency per descriptor and
        # trickle at ~30GB/s; 1-2KB rows stream at full rate.
        bi_sb = persist.tile([128, 132], BF16, tag="bi")      # ident | biases
        wqk_sb = persist.tile([128, 2, NMC, DK], X_DT, tag="wqk")  # wk | wq
        wv_sb = persist.tile([128, NMC, DV], BF16, tag="wv")
        nc.sync.dma_start(bi_sb[:], aps["pack_bi"][:])
        nc.sync.dma_start(wqk_sb[:], aps["pack_qk"][:])
        nc.scalar.dma_start(wv_sb[:], aps["wv"][:])
        ident_sb = bi_sb[:, 0:128]
        bias_f = persist.tile([128, 3], F32, tag="biasf")
        nc.vector.tensor_copy(bias_f[:], bi_sb[:, 128:131])
        bq_ap, bk_ap, bv_ap = (bias_f[:, 0:1], bias_f[:, 1:2],
                               bias_f[:, 2:3])
        wk_sb, wq_sb = wqk_sb[:, 0], wqk_sb[:, 1]
        ones_bf = persist.tile([128, 1], BF16, tag="ones")
        nc.vector.memset(ones_bf[:], 1.0)

        # --- persistent activations ---
        qpT = [persist.tile([128, 512], BF16, tag=f"qpT{i}", name=f"qpT{i}")
               for i in range(NQB)]
        kpT = [persist.tile([128, 512], BF16, tag=f"kpT{i}", name=f"kpT{i}")
               for i in range(NKB)]
        vp_pair = [persist.tile([128, 256], BF16, tag=f"vp{i}", name=f"vp{i}")
                   for i in range(NKC // 2)]
        sums_sb = persist.tile([1, S], F32, tag="sums")

        with (
            tc.tile_pool(name="spool", bufs=2, space="PSUM") as spool,
            tc.tile_pool(name="avp", bufs=1, space="PSUM") as avp,
            tc.tile_pool(name="pp", bufs=2, space="PSUM") as pp,
            tc.tile_pool(name="kxp", bufs=3) as kxp,
            tc.tile_pool(name="qxp", bufs=3) as qxp,
            tc.tile_pool(name="vxp", bufs=3) as vxp,
            tc.tile_pool(name="ep", bufs=14) as ep,
            tc.tile_pool(name="tmpp", bufs=3) as tmpp,
            tc.tile_pool(name="t2p", bufs=2) as t2p,
            tc.tile_pool(name="t3p", bufs=3) as t3p,
            tc.tile_pool(name="outp", bufs=2) as outp,
        ):
            # ---- input fetches (staged: early issues kept minimal so the
            # critical kx0/qx0/qx1 stream at full bandwidth) ----
            kxs, vxs, qxs = {}, {}, {}

            def fetch_k(i, split=False):
                t = kxp.tile([128, NMC, 512], X_DT, tag="kx",
                             name=f"kx{i}", bufs=NKB)
                if split:
                    h = NMC // 2
                    nc.sync.dma_start(t[:, 0:h, :], aps["kT"][i][:, 0:h, :])
                    nc.sync.dma_start(t[:, h:NMC, :], aps["kT"][i][:, h:NMC, :])
                else:
                    nc.sync.dma_start(t[:], aps["kT"][i])
                kxs[i] = t

            def fetch_v(i):
                t = vxp.tile([128, NMC, 512], BF16, tag="vx",
                             name=f"vx{i}", bufs=3)
                nc.sync.dma_start(t[:], aps["vT"][i])
                vxs[i] = t

            def fetch_q(i, split=False):
                t = qxp.tile([128, NMC, 512], X_DT, tag="qx",
                             name=f"qx{i}", bufs=4)
                if split:
                    h = NMC // 2
                    nc.gpsimd.dma_start(t[:, 0:h, :], aps["qT"][i][:, 0:h, :])
                    nc.gpsimd.dma_start(t[:, h:NMC, :], aps["qT"][i][:, h:NMC, :])
                else:
                    nc.gpsimd.dma_start(t[:], aps["qT"][i])
                qxs[i] = t

            # ---- projections ----
            # blocks are projected in PAIRS sharing each weight load: the PE
            # hides LDWEIGHTS only when >=2 matmuls consume one stationary
            # load (solo DR matmuls measured ~380ns vs 241ns paired).
            def proj_qk(xs_, w_sb, dsts, scale, bias_ap, name):
                pss = [pp.tile([128, 512], F32, tag="pp", name=f"ps_{name}{j}")
                       for j in range(len(xs_))]
                if USE_FP8:
                    for c in range(0, NMC, 2):
                        for ps, x in zip(pss, xs_):
                            nc.tensor.matmul(
                                ps[:], lhsT=w_sb[:, c:c + 2, :],
                                rhs=x[:, c:c + 2, :],
                                start=(c == 0), stop=(c == NMC - 2),
                                perf_mode=DoubleRow,
                            )
                else:
                    for c in range(NMC):
                        for ps, x in zip(pss, xs_):
                            nc.tensor.matmul(
                                ps[:], lhsT=w_sb[:, c, :], rhs=x[:, c, :],
                                start=(c == 0), stop=(c == NMC - 1),
                            )
                for ps, dst in zip(pss, dsts):
                    nc.vector.tensor_scalar(
                        dst[:], ps[:], scale, bias_ap,
                        op0=mybir.AluOpType.mult, op1=mybir.AluOpType.add,
                    )

            def proj_q(*qbs):
                proj_qk([qxs.pop(b) for b in qbs], wq_sb,
                        [qpT[b] for b in qbs],
                        (1.0 / W_SCALE) * SC if USE_FP8 else SC,
                        bq_ap, "q" + "".join(map(str, qbs)))

            def proj_k(*kbs):
                proj_qk([kxs.pop(b) for b in kbs], wk_sb,
                        [kpT[b] for b in kbs],
                        (1.0 / W_SCALE) if USE_FP8 else 1.0,
                        bk_ap, "k" + "".join(map(str, kbs)))

            def proj_v(*kbs):
                vxl = [vxs.pop(b) for b in kbs]
                pss = [pp.tile([128, 512], F32, tag="pp", name=f"ps_v{b}")
                       for b in kbs]
                for c in range(NMC):
                    for ps, vx in zip(pss, vxl):
                        nc.tensor.matmul(
                            ps[:], lhsT=wv_sb[:, c, :], rhs=vx[:, c, :],
                            start=(c == 0), stop=(c == NMC - 1),
                        )
                for kb, ps in zip(kbs, pss):
                    vpt = tmpp.tile([128, 512], BF16, tag="vpt",
                                    name=f"vpt{kb}")
                    nc.vector.tensor_scalar(
                        vpt[:], ps[:], 1.0, bv_ap,
                        op0=mybir.AluOpType.mult, op1=mybir.AluOpType.add,
                    )
                    for j in range(2):
                        tp = pp.tile([128, 256], BF16, tag="pp",
                                     name=f"tp{kb}_{j}")
                        for i in range(2):
                            nc.tensor.transpose(
                                tp[:, i * 128:(i + 1) * 128],
                                vpt[:, (2 * j + i) * 128:(2 * j + i + 1) * 128],
                                ident_sb,
                            )
                        nc.vector.tensor_copy(vp_pair[2 * kb + j][:], tp[:])

            # ---- attention machinery ----
            av_q = []           # pending (state, kc) AV emissions
            vp_ready = set()    # kb indices whose vp tiles are emitted

            def av_drain(n):
                k = 0
                while av_q and k < n:
                    st, kc = av_q[0]
                    if kc // 4 not in vp_ready:
                        break
                    av_q.pop(0)
                    k += 1
                    e1, half = st["es"][kc], kc % 2
                    vps = vp_pair[kc // 2][:, half * 128:(half + 1) * 128]
                    if st["av"] is None:
                        st["av"] = avp.tile([128, 1024], F32, tag="av",
                                            name=f"av{st['p']}")
                    for h in range(2):
                        nc.tensor.matmul(
                            st["av"][:, h * 512:(h + 1) * 512], lhsT=vps,
                            rhs=e1[:, h * 512:(h + 1) * 512],
                            start=(kc == 0), stop=(kc == NKC - 1),
                        )
                    if kc == NKC - 1:
                        finish_pair(st)

            def finish_pair(st):
                p = st["p"]
                outsb = outp.tile([128, 1024], BF16, tag="out", name=f"out{p}")
                for h in range(2):
                    nc.vector.tensor_copy(outsb[:, h * 512:(h + 1) * 512],
                                          st["av"][:, h * 512:(h + 1) * 512])
                    nc.gpsimd.dma_start(
                        aps["outT"][:, p * 1024 + h * 512:
                                    p * 1024 + (h + 1) * 512],
                        outsb[:, h * 512:(h + 1) * 512])

            def score_chunk(st, kc):
                p = st["p"]
                s1 = spool.tile([128, 1024], F32, tag="s", name=f"s{p}_{kc}")
                kslice = kpT[kc // 4][:, (kc % 4) * 128:(kc % 4 + 1) * 128]
                for h in range(2):
                    nc.tensor.matmul(
                        s1[:, h * 512:(h + 1) * 512],
                        lhsT=kslice, rhs=qpT[2 * p + h][:],
                        start=True, stop=True,
                    )
                e1 = ep.tile([128, 1024], BF16, tag="e", name=f"e{p}_{kc}")
                nc.scalar.activation(e1[:], s1[:], Exp)
                st["es"][kc] = e1
                av_q.append((st, kc))

            def sum_tree(st, sg):
                # bf16 reduction tree over exp tiles; per-partition rounding
                # noise washes out in the 128-way ones-matmul reduce.
                p = st["p"]
                ea, eb = st["es"][2 * sg], st["es"][2 * sg + 1]
                tmp = tmpp.tile([128, 1024], BF16, tag="tmp", name=f"t{p}_{sg}")
                nc.vector.tensor_add(tmp[:], ea[:], eb[:])
                if p == NPAIR - 1:
                    # last pair: accumulate the ones-matmul per score-group
                    # so the kernel tail is not a serial L2/L3 DVE cascade
                    if st["ps_sums"] is None:
                        st["ps_sums"] = [
                            pp.tile([1, 512], F32, tag="pp", name=f"sum{p}_{h}")
                            for h in range(2)]
                    for h in range(2):
                        nc.tensor.matmul(
                            st["ps_sums"][h][:], lhsT=ones_bf[:],
                            rhs=tmp[:, h * 512:(h + 1) * 512],
                            start=(sg == 0), stop=(sg == NSG - 1),
                        )
                    return
                st["t1"].append(tmp)
                if len(st["t1"]) == 2:
                    a, b = st["t1"]
                    st["t1"] = []
                    t2 = t2p.tile([128, 1024], BF16, tag="t2",
                                  name=f"t2_{p}_{sg}")
                    nc.vector.tensor_add(t2[:], a[:], b[:])
                    st["t2"].append(t2)
                if len(st["t2"]) == 2:
                    a, b = st["t2"]
                    st["t2"] = []
                    t3 = t3p.tile([128, 1024], BF16, tag="t3",
                                  name=f"t3_{p}_{sg}")
                    eng = nc.vector if p == NPAIR - 1 else nc.gpsimd
                    eng.tensor_add(t3[:], a[:], b[:])
                    st["t3"].append(t3)

            def pair_sums(st):
                p = st["p"]
                if st["ps_sums"] is not None:
                    ps_sums = st["ps_sums"]
                else:
                    assert len(st["t3"]) == 2
                    ps_sums = [pp.tile([1, 512], F32, tag="pp",
                                       name=f"sum{p}_{h}") for h in range(2)]
                    for ti, t3 in enumerate(st["t3"]):
                        for h in range(2):
                            nc.tensor.matmul(
                                ps_sums[h][:], lhsT=ones_bf[:],
                                rhs=t3[:, h * 512:(h + 1) * 512],
                                start=(ti == 0), stop=(ti == 1),
                            )
                for h in range(2):
                    nc.vector.tensor_copy(
                        sums_sb[:, p * 1024 + h * 512:p * 1024 + (h + 1) * 512],
                        ps_sums[h][:])
                nc.scalar.dma_start(
                    aps["sums"][:, p * 1024:(p + 1) * 1024],
                    sums_sb[:, p * 1024:(p + 1) * 1024])

            # proj + deferred-fetch work interleaved into (pass, score-group)
            # slots.  vp(kb) must precede AV chunk 4kb (drain checks
            # vp_ready); kp(b) must precede score chunk 4b; qp(2p,2p+1)
            # before pass p.
            def vstep(*kbs):
                def f():
                    proj_v(*kbs)
                    vp_ready.update(kbs)
                return f

            plan = {
                (0, 0): [lambda: fetch_q(2), lambda: fetch_q(3),
                         lambda: fetch_k(2), lambda: fetch_k(3),
                         lambda: proj_k(1)],
                (0, 1): [lambda: fetch_v(1), vstep(0)],
                (0, 2): [lambda: fetch_q(4), lambda: fetch_q(5)],
                (0, 3): [lambda: fetch_v(2), lambda: proj_k(2, 3)],
                (0, 4): [lambda: fetch_q(6), lambda: proj_q(2, 3)],
                (0, 5): [lambda: fetch_v(3), lambda: fetch_q(7),
                         vstep(1, 2)],
                (0, 6): [vstep(3)],
                (0, 7): [lambda: proj_q(4, 5)],
                (1, 0): [lambda: proj_q(6, 7)],
            }

            # ---- main schedule ----
            fetch_k(0, split=True)
            fetch_q(0, split=True)
            fetch_q(1, split=True)
            fetch_k(1)
            fetch_v(0)
            proj_k(0)
            proj_q(0, 1)
            states = [dict(p=p, es={}, av=None, t1=[], t2=[], t3=[],
                           ps_sums=None)
                      for p in range(NPAIR)]
            for p in range(NPAIR):
                st = states[p]
                for kc in range(NKC):
                    score_chunk(st, kc)
                    if kc % 2 == 1:
                        sg = kc // 2
                        for job in plan.get((p, sg), []):
                            job()
                        # DVE tree adds come after proj bias-adds (emission
                        # order is engine FIFO order; the tree is not urgent)
                        sum_tree(st, sg)
                    # drain AV down to AV_LAG chunks of backlog, at most 3
                    # per step so a burst never starves the exp stream
                    av_drain(min(3, max(0, len(av_q) - AV_LAG)))
                pair_sums(st)
            av_drain(len(av_q))


_CACHE = {}


def _build():
    if "nc" in _CACHE:
        return _CACHE["nc"]
    nc = bacc.Bacc("TRN2", debug=False, num_devices=N_CORES)
    aps = {
        "qT": nc.dram_tensor("qT", [NQB, 128, NMC, 512], X_DT,
                             kind="ExternalInput").ap(),
        "kT": nc.dram_tensor("kT", [NKB, 128, NMC, 512], X_DT,
                             kind="ExternalInput").ap(),
        "vT": nc.dram_tensor("vT", [NKB, 128, NMC, 512], BF16,
                             kind="ExternalInput").ap(),
        "pack_qk": nc.dram_tensor("pack_qk", [128, 2, NMC, DK], X_DT,
                                  kind="ExternalInput").ap(),
        "wv": nc.dram_tensor("wv", [128, NMC, DV], BF16,
                             kind="ExternalInput").ap(),
        "pack_bi": nc.dram_tensor("pack_bi", [128, 132], BF16,
                                  kind="ExternalInput").ap(),
        "outT": nc.dram_tensor("outT", [DV, S], BF16,
                               kind="ExternalOutput").ap(),
        "sums": nc.dram_tensor("sums", [1, S], F32,
                               kind="ExternalOutput").ap(),
    }
    with tile.TileContext(nc) as tc:
        _emit(tc, aps)
    nc.compile()
    _CACHE["nc"] = nc
    return nc


def _pack_w(w, dt):
    # [DM, d] -> [128, NMC, d]  (chunk-major weight layout)
    return np.ascontiguousarray(
        np.asarray(w, np.float32).reshape(NMC, 128, -1)
        .transpose(1, 0, 2).astype(dt))


def _pack_x(xT, nblk, dt):
    # [DM, n] -> [nblk, 128, NMC, 512]  (contiguous per-stripe layout)
    return np.ascontiguousarray(
        np.asarray(xT, np.float32).reshape(NMC, 128, nblk, 512)
        .transpose(2, 1, 0, 3).astype(dt))


def make_in_maps(q, k, v, wq, bq, wk, bk, wv, bv):
    ws = W_SCALE if USE_FP8 else 1.0
    wq_p = _pack_w(np.asarray(wq, np.float32) * ws, NP_X)
    wk_p = _pack_w(np.asarray(wk, np.float32) * ws, NP_X)
    wv_p = _pack_w(wv, NP_BF16)
    pack_qk = np.ascontiguousarray(np.stack([wk_p, wq_p], axis=1))
    pack_bi = np.zeros((128, 132), NP_BF16)
    pack_bi[:, 0:128] = np.eye(128, dtype=NP_BF16)
    pack_bi[:, 128] = (np.asarray(bq, np.float32) * SC).astype(NP_BF16)
    pack_bi[:, 129] = np.asarray(bk, np.float32).astype(NP_BF16)
    pack_bi[:, 130] = np.asarray(bv, np.float32).astype(NP_BF16)

    in_maps = []
    for core in range(N_CORES):
        b, h = core // 2, core % 2
        qTb = _pack_x(np.asarray(q[b], np.float32).T, NQB, NP_X)
        kTb = _pack_x(
            np.asarray(k[b], np.float32).T[:, h * SK:(h + 1) * SK], NKB, NP_X)
        vTb = _pack_x(
            np.asarray(v[b], np.float32).T[:, h * SK:(h + 1) * SK], NKB,
            NP_BF16)
        in_maps.append({
            "qT": qTb, "kT": kTb, "vT": vTb,
            "pack_qk": pack_qk, "wv": wv_p, "pack_bi": pack_bi,
        })
    return in_maps


def kernel(q, k, v, wq, bq, wk, bk, wv, bv, _trace=False, _tmpdir=None):
    nc = _build()
    in_maps = make_in_maps(q, k, v, wq, bq, wk, bk, wv, bv)
    res = run_bass_kernel_spmd(
        nc, in_maps, list(range(N_CORES)), trace=_trace, tmpdir=_tmpdir
    )
    out = np.empty((B, S, DV), np.float32)
    for b in range(B):
        rA, rB = res.results[2 * b], res.results[2 * b + 1]
        num = rA["outT"].astype(np.float32) + rB["outT"].astype(np.float32)
        den = rA["sums"] + rB["sums"]
        out[b] = (num / den).T
    if _trace:
        kernel.last_results = res
    return out
